# revision 1
# baseline (speedup 1.0000x reference)
"""Trainium2 Bass kernel for CustomRoPEAttention (B=2, S=2048, H=16, Dh=128).

Sharding: 8 cores = 2 batches x 4 head-groups (4 heads/core), tensor-parallel
over heads + data-parallel over batch. Each core computes QKV projection for
its heads (f32r matmuls), RoPE, causal softmax attention, and a partial
(transposed) output projection. Host sums the 4 partials per batch + bias.

Self-contained: hardcodes shapes from the problem spec.
"""
import math
from contextlib import ExitStack

import numpy as np

import concourse.mybir as mybir
import concourse.tile as tile
from concourse import bacc
from concourse.bass_utils import run_bass_kernel_spmd
from concourse.masks import make_identity

S = 2048            # sequence
D = 2048            # hidden
NH = 16             # total heads
DH = 128            # head dim
HG = 4              # heads per core
GQ = HG * DH        # 512: per-core q/k/v feature width
B = 2
NCORES = 8
ROPE_THETA = 10000.0
SCALE = 1.0 / math.sqrt(DH)
NEG = -1.0e9
SLAB = 256          # phase-1 sequence slab width
F32 = mybir.dt.float32
F32R = mybir.dt.float32r
MULT = mybir.AluOpType.mult
ADD = mybir.AluOpType.add


def build_nc(reps=1, phases=(1, 2, 3), knobs=None):
    kn = {"p1x": 4, "p1s": 4, "p2a": 4, "p2t": 2, "p2sp": 2, "p2tp": 3, "p2cp": 1, "p3ps": 4, "spw": 1024, "slab": SLAB, "p1ps": 3, "p1vps": 2, "splitw": 0, "vfirst": 0, "norope": 0, "nospill": 0, "atb": 4, "aev": 0, "wo_early": 1}
    if knobs:
        kn.update(knobs)
    nc = bacc.Bacc(None, target_bir_lowering=False)
    xt = nc.dram_tensor("xt", [16, 128, S], F32R, kind="ExternalInput")       # x^T tiles [kc,p,s]
    wqk = nc.dram_tensor("wqk", [16, 128, 2 * GQ], F32R, kind="ExternalInput")
    wv = nc.dram_tensor("wv", [16, 128, GQ], F32R, kind="ExternalInput")
    wo = nc.dram_tensor("wo", [4, 128, D], F32R, kind="ExternalInput")        # Wo rows tiles
    bqkt = nc.dram_tensor("bqkt", [128, 8], F32, kind="ExternalInput")        # q/k bias per (dh, mt)
    bqkt_sw = nc.dram_tensor("bqkt_sw", [128, 8], F32, kind="ExternalInput")   # same, halves swapped
    bv = nc.dram_tensor("bv", [1, GQ], F32, kind="ExternalInput")
    cost = nc.dram_tensor("cost", [128, S], F32, kind="ExternalInput")        # cos^T
    sinrt = nc.dram_tensor("sinrt", [128, S], F32, kind="ExternalInput")      # sin^T with rot sign
    maskd = nc.dram_tensor("maskd", [128, 128], F32, kind="ExternalInput")    # diag causal add-mask
    outt = nc.dram_tensor("outt", [16, 128, S], F32, kind="ExternalOutput")   # partial^T tiles
    qks = nc.dram_tensor("qks", [2 * HG, 128, S], F32R)                       # spill: q then k head tiles
    vsp = nc.dram_tensor("vsp", [16, 128, GQ], F32R)                          # spill: V natural tiles

    with tile.TileContext(nc) as tc, ExitStack() as top:
        g = top.enter_context(tc.tile_pool(name="glob", bufs=1))
        tcos = g.tile([128, S], F32)
        nc.sync.dma_start(out=tcos, in_=cost[:])
        tsin = g.tile([128, S], F32)
        nc.sync.dma_start(out=tsin, in_=sinrt[:])
        tmask = g.tile([128, 128], F32)
        nc.sync.dma_start(out=tmask, in_=maskd[:])
        ident_f = g.tile([128, 128], F32)
        make_identity(nc, ident_f[:])
        ident = g.tile([128, 128], F32R)
        nc.vector.tensor_copy(out=ident[:], in_=ident_f[:])
        tbqkt = g.tile([128, 8], F32)
        nc.sync.dma_start(out=tbqkt, in_=bqkt[:])
        tbqkt_sw = g.tile([128, 8], F32)
        nc.sync.dma_start(out=tbqkt_sw, in_=bqkt_sw[:])
        tbvb = g.tile([128, GQ], F32)
        nc.sync.dma_start(out=tbvb, in_=bv[:].to_broadcast((128, GQ)))
        for _rep in range(reps):
          if 1 in phases:
            # ---------------- Phase 1: QKV^T projection + RoPE + spill ----------------
            SLB = kn["slab"]
            with tc.tile_pool(name="p1w", bufs=1) as p1w, \
                 tc.tile_pool(name="p1x", bufs=kn["p1x"]) as p1x, \
                 tc.tile_pool(name="p1s", bufs=kn["p1s"]) as p1s, \
                 tc.tile_pool(name="p1ps", bufs=kn["p1ps"], space="PSUM") as p1ps, \
                 tc.tile_pool(name="p1vps", bufs=kn["p1vps"], space="PSUM") as p1vps:
                twqk = []
                twv = []
                for _kc in range(16):
                    wqkt = p1w.tile([128, 2 * GQ], F32R, tag=f"wqk{_kc}")
                    nc.sync.dma_start(out=wqkt, in_=wqk[_kc])
                    twqk.append(wqkt)
                    wvt = p1w.tile([128, GQ], F32R, tag=f"wv{_kc}")
                    nc.sync.dma_start(out=wvt, in_=wv[_kc])
                    twv.append(wvt)
                for ns in range(S // SLB):
                    sl = slice(ns * SLB, (ns + 1) * SLB)
                    xs = p1x.tile([128, 16, SLB], F32R, tag="xs")
                    nc.sync.dma_start(out=xs, in_=xt[:, :, sl].rearrange("kc p s -> p kc s"))
                    # Q^T and K^T head tiles (mt 0..3 = q heads, 4..7 = k heads)
                    for mt in range(2 * HG):
                        pqk = p1ps.tile([128, SLB], F32, tag="qkps")
                        for kc in range(16):
                            nc.tensor.matmul(pqk[:], twqk[kc][:, mt * 128:(mt + 1) * 128],
                                             xs[:, kc, :], start=(kc == 0), stop=(kc == 15))
                        if kn["norope"]:
                            qf = p1s.tile([128, SLB], F32R, tag="qf")
                            nc.scalar.copy(out=qf[:], in_=pqk[:])
                        else:
                            qraw = p1s.tile([128, SLB], F32, tag="qraw")
                            nc.scalar.copy(out=qraw[:], in_=pqk[:])
                            qsw = p1s.tile([128, SLB], F32, tag="qsw")
                            nc.sync.dma_start(out=qsw[0:64, :], in_=qraw[64:128, :])
                            nc.sync.dma_start(out=qsw[64:128, :], in_=qraw[0:64, :])
                            m1 = p1s.tile([128, SLB], F32R, tag="m1")
                            nc.vector.scalar_tensor_tensor(
                                out=m1[:], in0=pqk[:], scalar=tbqkt[:, mt:mt + 1],
                                in1=tcos[:, sl], op0=ADD, op1=MULT)
                            m2 = p1s.tile([128, SLB], F32R, tag="m2")
                            nc.vector.scalar_tensor_tensor(
                                out=m2[:], in0=qsw[:], scalar=tbqkt_sw[:, mt:mt + 1],
                                in1=tsin[:, sl], op0=ADD, op1=MULT)
                            qf = p1s.tile([128, SLB], F32R, tag="qf")
                            nc.vector.tensor_tensor(out=qf[:], in0=m1[:], in1=m2[:], op=ADD)
                        if not kn["nospill"]:
                            nc.sync.dma_start(out=qks[mt, :, sl], in_=qf[:])
                    # V natural tiles for this slab
                    for st in range(SLB // 128):
                        pv = p1vps.tile([128, GQ], F32, tag="vps")
                        s0 = st * 128
                        for kc in range(16):
                            nc.tensor.matmul(pv[:], xs[:, kc, s0:s0 + 128],
                                             twv[kc][:], start=(kc == 0), stop=(kc == 15))
                        vsb = p1s.tile([128, GQ], F32R, tag="vsb")
                        nc.vector.tensor_tensor(out=vsb[:], in0=pv[:], in1=tbvb[:], op=ADD)
                        if not kn["nospill"]:
                            nc.sync.dma_start(out=vsp[ns * (SLB // 128) + st], in_=vsb[:])

          # C^T [ (head,dh), S ] persists from phase 2 into phase 3
          ctstack = ExitStack()
          ctpool = ctstack.enter_context(tc.tile_pool(name="ctp", bufs=1))
          two_early = None
          if kn["wo_early"]:
              two_early = ctpool.tile([128, 4, D], F32R, tag="two_early")
              nc.sync.dma_start(out=two_early, in_=wo.rearrange("kc p f -> p kc f"))
          ct_sb = {}
          for _h in range(HG):
              for _q in range(4):
                  ctq = ctpool.tile([128, 512], F32R, tag=f"ct_{_h}_{_q}")
                  ct_sb[(_h, _q)] = ctq
          if 2 in phases:
            # ---------------- Phase 2: attention per head ----------------
            with tc.tile_pool(name="p2h", bufs=2) as p2h, \
                 tc.tile_pool(name="p2a", bufs=kn["p2a"]) as p2a, \
                 tc.tile_pool(name="p2t", bufs=kn["p2t"]) as p2t, \
                 tc.tile_pool(name="p2sp", bufs=kn["p2sp"], space="PSUM") as p2sp, \
                 tc.tile_pool(name="p2tp", bufs=kn["p2tp"], space="PSUM") as p2tp, \
                 tc.tile_pool(name="p2cp", bufs=kn["p2cp"], space="PSUM") as p2cp:
                for h in range(HG):
                    qh2, kh2, vh2 = [], [], []
                    for half in range(2):
                        qht = p2h.tile([128, 1024], F32R, tag=f"qh{half}")
                        nc.sync.dma_start(out=qht, in_=qks[h][:, half * 1024:(half + 1) * 1024])
                        qh2.append(qht)
                        kht = p2h.tile([128, 1024], F32R, tag=f"kh{half}")
                        nc.sync.dma_start(out=kht, in_=qks[HG + h][:, half * 1024:(half + 1) * 1024])
                        kh2.append(kht)
                        vht = p2h.tile([128, 8, 128], F32R, tag=f"vh{half}")
                        nc.sync.dma_start(
                            out=vht,
                            in_=vsp[half * 8:(half + 1) * 8, :,
                                    h * 128:(h + 1) * 128].rearrange("t p f -> p t f"))
                        vh2.append(vht)
                    for j in range(8):
                        at_sb = p2t.tile([128, 16, 256], F32R, tag="atsb")
                        for ii, i in enumerate((2 * j, 2 * j + 1)):
                            ski = (i + 1) * 128
                            spw = kn["spw"]
                            nchunk = (ski + spw - 1) // spw
                            ai = p2a.tile([128, S], F32R, tag="ai")
                            hs = p2a.tile([128, 4], F32, tag="hs")
                            for cc in range(nchunk):
                                off = cc * spw
                                w = min(spw, ski - off)
                                sp = p2sp.tile([128, spw], F32, tag="sp")
                                for s5 in range(0, w, 512):
                                    w5 = min(512, w - s5)
                                    ko = off + s5
                                    nc.tensor.matmul(
                                        sp[:, s5:s5 + w5],
                                        qh2[i // 8][:, (i % 8) * 128:(i % 8 + 1) * 128],
                                        kh2[ko // 1024][:, ko % 1024:ko % 1024 + w5],
                                        start=True, stop=True)
                                if off <= i * 128 < off + w:  # diagonal block lives here
                                    dd = i * 128 - off
                                    nc.vector.tensor_tensor(out=sp[:, dd:dd + 128],
                                                            in0=sp[:, dd:dd + 128],
                                                            in1=tmask[:], op=ADD)
                                nc.scalar.activation(out=ai[:, off:off + w], in_=sp[:, 0:w],
                                                     func=mybir.ActivationFunctionType.Exp,
                                                     scale=SCALE, accum_out=hs[:, cc:cc + 1])
                            for cc in range(1, nchunk):
                                nc.vector.tensor_tensor(out=hs[:, 0:1], in0=hs[:, 0:1],
                                                        in1=hs[:, cc:cc + 1], op=ADD)
                            rec = p2a.tile([128, 1], F32, tag="rec")
                            nc.vector.reciprocal(out=rec[:], in_=hs[:, 0:1])
                            nc.vector.tensor_tensor(out=ai[:, 0:ski], in0=ai[:, 0:ski],
                                                    in1=rec[:].broadcast_to((128, ski)), op=MULT)
                            ATB = kn["atb"]
                            for ks0 in range(0, i + 1, ATB):
                                nb = min(ATB, i + 1 - ks0)
                                atp = p2tp.tile([128, ATB, 128], F32R, tag="atp")
                                for t in range(nb):
                                    nc.tensor.transpose(atp[:, t, :],
                                                        ai[:, (ks0 + t) * 128:(ks0 + t + 1) * 128],
                                                        ident[:])
                                dst = at_sb[:, ks0:ks0 + nb, ii * 128:(ii + 1) * 128]
                                use_act = (kn["aev"] == 2 or
                                           (kn["aev"] == 0 and (ks0 // ATB + ii) % 2 == 0))
                                if use_act:
                                    nc.scalar.copy(out=dst, in_=atp[:, 0:nb, :])
                                else:
                                    nc.vector.tensor_copy(out=dst, in_=atp[:, 0:nb, :])
                        ct = p2cp.tile([128, 256], F32, tag="ct")
                        for ks in range(2 * j + 1):
                            nc.tensor.matmul(ct[:], vh2[ks // 8][:, ks % 8, :],
                                             at_sb[:, ks, :],
                                             start=(ks == 0), stop=False)
                        ksl = 2 * j + 1
                        nc.tensor.matmul(ct[:, 128:256], vh2[ksl // 8][:, ksl % 8, :],
                                         at_sb[:, ksl, 128:256], start=False, stop=True)
                        nc.scalar.copy(
                            out=ct_sb[(h, j // 2)][:, (j % 2) * 256:(j % 2 + 1) * 256],
                            in_=ct[:])

          if 3 in phases:
            # ---------------- Phase 3: output projection (transposed partial) ----------------
            with tc.tile_pool(name="p3w", bufs=1) as p3w, \
                 tc.tile_pool(name="p3s", bufs=4) as p3s, \
                 tc.tile_pool(name="p3ps", bufs=kn["p3ps"], space="PSUM") as p3ps:
                if two_early is not None:
                    two = two_early
                else:
                    two = p3w.tile([128, 4, D], F32R)
                    nc.sync.dma_start(out=two, in_=wo.rearrange("kc p f -> p kc f"))
                for ncc in range(4):
                    for mt in range(16):
                        op = p3ps.tile([128, 512], F32, tag="op")
                        for kh in range(4):
                            nc.tensor.matmul(op[:], two[:, kh, mt * 128:(mt + 1) * 128],
                                             ct_sb[(kh, ncc)][:],
                                             start=(kh == 0), stop=(kh == 3))
                        ob = p3s.tile([128, 512], F32, tag="ob")
                        if mt % 2 == 0:
                            nc.vector.tensor_copy(out=ob[:], in_=op[:])
                        else:
                            nc.scalar.copy(out=ob[:], in_=op[:])
                        nc.sync.dma_start(out=outt[mt, :, ncc * 512:(ncc + 1) * 512], in_=ob[:])
          ctstack.close()
    nc.finalize()
    return nc


_NC_CACHE = {}


def _get_nc(reps=1):
    if reps not in _NC_CACHE:
        _NC_CACHE[reps] = build_nc(reps)
    return _NC_CACHE[reps]


def _rope_tables(position_ids_b):
    pos = position_ids_b.astype(np.float32)
    inv_freq = (1.0 / (ROPE_THETA ** (np.arange(0, DH, 2, dtype=np.float32) / np.float32(DH))))
    ang = pos[:, None] * inv_freq[None, :]          # [S, 64]
    emb = np.concatenate([ang, ang], axis=-1)       # [S, 128]
    cosT = np.ascontiguousarray(np.cos(emb).T)      # [128, S]
    sinT = np.sin(emb).T
    sin_rot = np.concatenate([-sinT[0:64], sinT[64:128]], axis=0)
    return cosT.astype(np.float32), np.ascontiguousarray(sin_rot).astype(np.float32)


def _make_in_maps(inputs):
    hidden_states = np.asarray(inputs["hidden_states"], dtype=np.float32)
    position_ids = np.asarray(inputs["position_ids"])
    Wqkv = np.asarray(inputs["Wqkv"], dtype=np.float32)
    bqkv = np.asarray(inputs["bqkv"], dtype=np.float32)
    Wo = np.asarray(inputs["Wo"], dtype=np.float32)

    mask = np.triu(np.full((128, 128), NEG, dtype=np.float32), k=1)
    tabs = [_rope_tables(np.asarray(position_ids)[b]) for b in range(B)]
    xts = [np.ascontiguousarray(hidden_states[b].T).reshape(16, 128, S) for b in range(B)]

    in_maps = []
    for c in range(NCORES):
        b, hg = divmod(c, HG)
        qcols = slice(hg * GQ, (hg + 1) * GQ)
        kcols = slice(D + hg * GQ, D + (hg + 1) * GQ)
        vcols = slice(2 * D + hg * GQ, 2 * D + (hg + 1) * GQ)
        wqk_c = np.ascontiguousarray(
            np.concatenate([Wqkv[:, qcols], Wqkv[:, kcols]], axis=1)).reshape(16, 128, 2 * GQ)
        wv_c = np.ascontiguousarray(Wqkv[:, vcols]).reshape(16, 128, GQ)
        wo_c = np.ascontiguousarray(Wo[hg * GQ:(hg + 1) * GQ, :]).reshape(4, 128, D)
        bqk_c = np.concatenate([bqkv[qcols], bqkv[kcols]]).reshape(8, 128).T
        bqk_sw = np.concatenate([bqk_c[64:128], bqk_c[0:64]], axis=0)
        bv_c = bqkv[vcols].reshape(1, GQ)
        cosT, sin_rot = tabs[b]
        in_maps.append({
            "xt": xts[b], "wqk": wqk_c, "wv": wv_c, "wo": wo_c,
            "bqkt": np.ascontiguousarray(bqk_c), "bqkt_sw": np.ascontiguousarray(bqk_sw),
            "bv": np.ascontiguousarray(bv_c),
            "cost": cosT, "sinrt": sin_rot, "maskd": mask,
        })
    return in_maps


def kernel(hidden_states, position_ids, Wqkv, bqkv, Wo, bo, _reps=1):
    bo = np.asarray(bo, dtype=np.float32)
    in_maps = _make_in_maps({
        "hidden_states": hidden_states, "position_ids": position_ids,
        "Wqkv": Wqkv, "bqkv": bqkv, "Wo": Wo, "bo": bo,
    })
    nc = _get_nc(_reps)
    res = run_bass_kernel_spmd(nc, in_maps, core_ids=list(range(NCORES)))

    out = np.empty((B, S, D), dtype=np.float32)
    for b in range(B):
        acc = res.results[b * HG]["outt"].reshape(D, S).astype(np.float32).copy()
        for hg in range(1, HG):
            acc += res.results[b * HG + hg]["outt"].reshape(D, S)
        out[b] = acc.T + bo[None, :]
    return out



# revision 8
# speedup vs baseline: 1.2170x; 1.2170x over previous
"""Trainium2 Bass kernel for CustomRoPEAttention (B=2, S=2048, H=16, Dh=128).

Sharding: 8 cores = 2 batches x 4 head-groups (4 heads/core). Each core:
QKV projection (bf16 matmuls) + RoPE for its heads, transposed-layout causal
attention (scores computed as S^T with keys on partitions so the attention
probabilities feed A@V directly as the moving operand -- no PE transposes),
softmax denominators via ap-size-1 matmuls + deferred normalization, and a
partial (transposed) output projection. Host sums the 4 partials per batch.

Self-contained: hardcodes shapes from the problem spec.
"""
import math
from contextlib import ExitStack

import numpy as np
import ml_dtypes

import concourse.mybir as mybir
import concourse.tile as tile
from concourse import bacc
from concourse.bass_utils import run_bass_kernel_spmd
from concourse.masks import make_identity

S = 2048            # sequence
D = 2048            # hidden
NH = 16             # total heads
DH = 128            # head dim
HG = 4              # heads per core
GQ = HG * DH        # 512: per-core q/k/v feature width
B = 2
NCORES = 8
ROPE_THETA = 10000.0
SCALE = 1.0 / math.sqrt(DH)
NEG = -1.0e9
SLAB = 512          # phase-1 sequence slab width
F32 = mybir.dt.float32
BF16 = mybir.dt.bfloat16
F16 = mybir.dt.float16
MULT = mybir.AluOpType.mult
ADD = mybir.AluOpType.add
NB = S // 128       # 16 k/q blocks


def build_nc(reps=1, knobs=None):
    kn = {"p1ps": 3, "p1vps": 2, "p1x": 2, "sps": 2, "avps": 1, "p3ps": 4}
    if knobs:
        kn.update(knobs)
    nc = bacc.Bacc(None, target_bir_lowering=False)
    xt = nc.dram_tensor("xt", [16, 128, S], BF16, kind="ExternalInput")
    wqk = nc.dram_tensor("wqk", [16, 128, 2 * GQ], BF16, kind="ExternalInput")
    wv = nc.dram_tensor("wv", [16, 128, GQ], BF16, kind="ExternalInput")
    wo = nc.dram_tensor("wo", [4, 128, D], BF16, kind="ExternalInput")
    bqkt = nc.dram_tensor("bqkt", [128, 8], F32, kind="ExternalInput")
    bqkt_sw = nc.dram_tensor("bqkt_sw", [128, 8], F32, kind="ExternalInput")
    bv = nc.dram_tensor("bv", [1, GQ], F32, kind="ExternalInput")
    cost = nc.dram_tensor("cost", [128, S], BF16, kind="ExternalInput")    # cos^T
    sinrt = nc.dram_tensor("sinrt", [128, S], BF16, kind="ExternalInput")  # sin^T, rot sign
    maskd = nc.dram_tensor("maskd", [128, 128], F32, kind="ExternalInput")  # tril(-1) NEG
    onesb = nc.dram_tensor("onesb", [128, 1], BF16, kind="ExternalInput")
    ones1 = nc.dram_tensor("ones1", [1, 128], F16, kind="ExternalInput")
    outt = nc.dram_tensor("outt", [16, 128, S], BF16, kind="ExternalOutput")
    lrt = nc.dram_tensor("lrt", [HG, 16, 128], F16)  # recip bounce: [16,128] -> [1,2048]

    with tile.TileContext(nc) as tc, ExitStack() as top:
        g = top.enter_context(tc.tile_pool(name="glob", bufs=1))
        tcos = g.tile([128, S], BF16)
        nc.sync.dma_start(out=tcos, in_=cost[:])
        tsin = g.tile([128, S], BF16)
        nc.sync.dma_start(out=tsin, in_=sinrt[:])
        tmask = g.tile([128, 128], F32)
        nc.sync.dma_start(out=tmask, in_=maskd[:])
        ident_f = g.tile([128, 128], F32)
        make_identity(nc, ident_f[:])
        identh = g.tile([128, 128], F16)
        nc.vector.tensor_copy(out=identh[:], in_=ident_f[:])
        tbqkt = g.tile([128, 8], F32)
        nc.sync.dma_start(out=tbqkt, in_=bqkt[:])
        tbqkt_sw = g.tile([128, 8], F32)
        nc.sync.dma_start(out=tbqkt_sw, in_=bqkt_sw[:])
        tbvb = g.tile([128, GQ], F32)
        nc.sync.dma_start(out=tbvb, in_=bv[:].to_broadcast((128, GQ)))
        tones = g.tile([128, 1], BF16)
        nc.sync.dma_start(out=tones, in_=onesb[:])
        tones1 = g.tile([1, 128], F16)
        nc.sync.dma_start(out=tones1, in_=ones1[:])

        # Whole-kernel residents
        res = top.enter_context(tc.tile_pool(name="res", bufs=1))
        qt = []  # mt 0..3 = Q^T heads, 4..7 = K^T heads, each [128(dh), S] bf16
        for mt in range(2 * HG):
            qt.append(res.tile([128, S], BF16, tag=f"qt{mt}", name=f"qt{mt}"))
        vres = []  # 16 V k-block tiles [128(seq), GQ] bf16
        for t in range(NB):
            vres.append(res.tile([128, GQ], BF16, tag=f"v{t}", name=f"v{t}"))
        two = res.tile([128, 4, D], BF16, tag="two")
        nc.sync.dma_start(out=two, in_=wo.rearrange("kc p f -> p kc f"))
        ct_sb = {}
        for h in range(HG):
            for gq in range(4):
                ct_sb[(h, gq)] = res.tile([128, 512], BF16, tag=f"ct_{h}_{gq}", name=f"ct_{h}_{gq}")

        for _rep in range(reps):
          # ---------------- Phase 1: QKV^T projection + RoPE (all resident) ----------
          with tc.tile_pool(name="p1w", bufs=1) as p1w, \
               tc.tile_pool(name="p1x", bufs=kn["p1x"]) as p1x, \
               tc.tile_pool(name="qswp", bufs=2) as qswp, \
               tc.tile_pool(name="p1stg", bufs=1) as p1stg, \
               tc.tile_pool(name="p1ps", bufs=kn["p1ps"], space="PSUM") as p1ps, \
               tc.tile_pool(name="p1vps", bufs=kn["p1vps"], space="PSUM") as p1vps:
            twqk = []
            twv = []
            for kc in range(16):
                wqkt = p1w.tile([128, 2 * GQ], BF16, tag=f"wqk{kc}")
                nc.sync.dma_start(out=wqkt, in_=wqk[kc])
                twqk.append(wqkt)
                wvt = p1w.tile([128, GQ], BF16, tag=f"wv{kc}")
                nc.sync.dma_start(out=wvt, in_=wv[kc])
                twv.append(wvt)
            for ns in range(S // SLAB):
                sl = slice(ns * SLAB, (ns + 1) * SLAB)
                xs = p1x.tile([128, 16, SLAB], BF16, tag="xs")
                nc.sync.dma_start(out=xs, in_=xt[:, :, sl].rearrange("kc p s -> p kc s"))
                for mt in range(2 * HG):
                    pqk = p1ps.tile([128, SLAB], F32, tag="qkps")
                    for kc in range(16):
                        nc.tensor.matmul(pqk[:], twqk[kc][:, mt * 128:(mt + 1) * 128],
                                         xs[:, kc, :], start=(kc == 0), stop=(kc == 15))
                    nc.scalar.copy(out=qt[mt][:, sl], in_=pqk[:])
                for st in range(SLAB // 128):
                    pv = p1vps.tile([128, GQ], F32, tag="vps")
                    s0 = st * 128
                    for kc in range(16):
                        nc.tensor.matmul(pv[:], xs[:, kc, s0:s0 + 128],
                                         twv[kc][:], start=(kc == 0), stop=(kc == 15))
                    nc.vector.tensor_tensor(out=vres[ns * (SLAB // 128) + st],
                                            in0=pv[:], in1=tbvb[:], op=ADD)
            # RoPE per head tile (full width): q' = (q+b)*cos + swap(q+b)*sin_rot
            for mt in range(2 * HG):
                qsw = qswp.tile([128, S], BF16, tag="qsw")
                nc.sync.dma_start(out=qsw[0:64, :], in_=qt[mt][64:128, :])
                nc.sync.dma_start(out=qsw[64:128, :], in_=qt[mt][0:64, :])
                m1 = p1stg.tile([128, S], BF16, tag="m1")
                nc.vector.scalar_tensor_tensor(
                    out=m1[:], in0=qt[mt][:], scalar=tbqkt[:, mt:mt + 1],
                    in1=tcos[:], op0=ADD, op1=MULT)
                m2 = p1stg.tile([128, S], BF16, tag="m2")
                nc.vector.scalar_tensor_tensor(
                    out=m2[:], in0=qsw[:], scalar=tbqkt_sw[:, mt:mt + 1],
                    in1=tsin[:], op0=ADD, op1=MULT)
                nc.vector.tensor_tensor(out=qt[mt][:], in0=m1[:], in1=m2[:], op=ADD)

          # ---------------- Phase 2: transposed attention ----------------
          p2stack = ExitStack()
          expp = p2stack.enter_context(tc.tile_pool(name="expp", bufs=2))
          lrp = p2stack.enter_context(tc.tile_pool(name="lrp", bufs=2))
          avps = p2stack.enter_context(
              tc.tile_pool(name="avps", bufs=kn["avps"], space="PSUM"))
          rbps = p2stack.enter_context(tc.tile_pool(name="rbps", bufs=1, space="PSUM"))
          p2inner = ExitStack()
          sps = p2inner.enter_context(
              tc.tile_pool(name="sps", bufs=kn["sps"], space="PSUM"))
          smps = p2inner.enter_context(tc.tile_pool(name="smps", bufs=1, space="PSUM"))

          expT = [None] * HG  # per live head: list of 16 exp(S^T) tiles
          recrow = [None] * HG

          def denom(h, b):
              # ell[q] for q-block b: sum_k exp tiles, ap-1 matmuls, then recip
              lp = smps.tile([128, 1], F32, tag="lp")
              for j in range(b + 1):
                  nc.tensor.matmul(lp[:], expT[h][j][:, (b - j) * 128:(b - j + 1) * 128],
                                   tones[:], start=(j == 0), stop=(j == b))
              with nc.allow_low_precision(reason="softmax recip in f16 is plenty"):
                  nc.vector.reciprocal(out=lrec_cur[h][:, b:b + 1], in_=lp[:])

          def sweep2_group(h, gq):
              # normalize+accumulate: ct = (sum_k V^T[k] expS^T[k]) * recip(ell)
              rb = rbps.tile([128, 512], F32, tag="rb")
              nc.tensor.matmul(rb[:], tones1[:], recrow[h][:, gq * 512:(gq + 1) * 512],
                               start=True, stop=True)
              ct = avps.tile([128, 512], F32, tag="ct")
              last = 4 * gq + 3
              for j in range(last + 1):
                  if j <= 4 * gq:
                      nc.tensor.matmul(ct[:], vres[j][:, h * 128:(h + 1) * 128],
                                       expT[h][j][:, (4 * gq - j) * 128:(4 * gq - j) * 128 + 512],
                                       start=(j == 0), stop=(j == last))
                  else:
                      w = (4 * gq + 4 - j) * 128
                      nc.tensor.matmul(ct[:, 512 - w:512], vres[j][:, h * 128:(h + 1) * 128],
                                       expT[h][j][:, 0:w], start=False, stop=(j == last))
              nc.vector.tensor_tensor(out=ct_sb[(h, gq)][:], in0=ct[:], in1=rb[:], op=MULT)

          lrec_cur = {}
          for h in range(HG):
              expT[h] = []
              lrec_cur[h] = lrp.tile([128, 16], F16, tag="lrec", name="lrec")
              recrow[h] = lrp.tile([1, S], F16, tag="recrow", name="recrow")
              for i in range(NB):
                  w = (NB - i) * 128
                  ex = expp.tile([128, w], BF16, tag=f"expT{i}", name=f"expT{i}")
                  expT[h].append(ex)
                  for c0 in range(0, w, 1024):
                      cw = min(1024, w - c0)
                      sp = sps.tile([128, 1024], F32, tag="sp")
                      for s5 in range(0, cw, 512):
                          w5 = min(512, cw - s5)
                          q0 = i * 128 + c0 + s5
                          nc.tensor.matmul(sp[:, s5:s5 + w5],
                                           qt[HG + h][:, i * 128:(i + 1) * 128],
                                           qt[h][:, q0:q0 + w5], start=True, stop=True)
                      if c0 == 0:
                          nc.vector.tensor_tensor(out=sp[:, 0:128], in0=sp[:, 0:128],
                                                  in1=tmask[:], op=ADD)
                      nc.scalar.activation(out=ex[:, c0:c0 + cw], in_=sp[:, 0:cw],
                                           func=mybir.ActivationFunctionType.Exp,
                                           scale=SCALE)
                  if i >= 2:
                      denom(h, i - 2)
                  if h >= 1 and i % 4 == 3:
                      sweep2_group(h - 1, i // 4)
              denom(h, NB - 2)
              denom(h, NB - 1)
              # recip row: [128,16] -> transpose -> [16,128] -> DRAM -> [1,2048]
              rt = smps.tile([16, 128], F16, tag="rt")
              nc.tensor.transpose(rt[:], lrec_cur[h][:], identh[:])
              rts = lrp.tile([16, 128], F16, tag="rts")
              nc.vector.tensor_copy(out=rts[:], in_=rt[:])
              nc.sync.dma_start(out=lrt[h], in_=rts[:])
              nc.sync.dma_start(out=recrow[h][:].rearrange("one s -> (one s)"),
                                in_=lrt[h].rearrange("a b -> (a b)"))

          # close S^T/denom psum pools before opening phase-3 psum
          p2inner.close()

          # ---------------- Phase 3: output projection, interleaving head-3 sweep2 ----
          with tc.tile_pool(name="p3s", bufs=4) as p3s, \
               tc.tile_pool(name="p3ps", bufs=kn["p3ps"], space="PSUM") as p3ps:
              for gq in range(4):
                  sweep2_group(HG - 1, gq)
                  for mt in range(16):
                      op = p3ps.tile([128, 512], F32, tag="op")
                      for kh in range(HG):
                          nc.tensor.matmul(op[:], two[:, kh, mt * 128:(mt + 1) * 128],
                                           ct_sb[(kh, gq)][:],
                                           start=(kh == 0), stop=(kh == 3))
                      ob = p3s.tile([128, 512], BF16, tag="ob")
                      if mt % 2 == 0:
                          nc.vector.tensor_copy(out=ob[:], in_=op[:])
                      else:
                          nc.scalar.copy(out=ob[:], in_=op[:])
                      nc.sync.dma_start(out=outt[mt, :, gq * 512:(gq + 1) * 512], in_=ob[:])
          p2stack.close()
    nc.finalize()
    return nc


_NC_CACHE = {}


def _get_nc(reps=1):
    if reps not in _NC_CACHE:
        _NC_CACHE[reps] = build_nc(reps)
    return _NC_CACHE[reps]


def _rope_tables(position_ids_b):
    pos = position_ids_b.astype(np.float32)
    inv_freq = (1.0 / (ROPE_THETA ** (np.arange(0, DH, 2, dtype=np.float32) / np.float32(DH))))
    ang = pos[:, None] * inv_freq[None, :]          # [S, 64]
    emb = np.concatenate([ang, ang], axis=-1)       # [S, 128]
    cosT = np.ascontiguousarray(np.cos(emb).T)      # [128, S]
    sinT = np.sin(emb).T
    sin_rot = np.concatenate([-sinT[0:64], sinT[64:128]], axis=0)
    return cosT.astype(ml_dtypes.bfloat16), np.ascontiguousarray(sin_rot).astype(ml_dtypes.bfloat16)


def _make_in_maps(inputs):
    hidden_states = np.asarray(inputs["hidden_states"], dtype=np.float32)
    position_ids = np.asarray(inputs["position_ids"])
    Wqkv = np.asarray(inputs["Wqkv"], dtype=np.float32)
    bqkv = np.asarray(inputs["bqkv"], dtype=np.float32)
    Wo = np.asarray(inputs["Wo"], dtype=np.float32)

    mask = np.tril(np.full((128, 128), NEG, dtype=np.float32), k=-1)
    tabs = [_rope_tables(np.asarray(position_ids)[b]) for b in range(B)]
    xts = [np.ascontiguousarray(hidden_states[b].T).astype(ml_dtypes.bfloat16)
           .reshape(16, 128, S) for b in range(B)]
    onesb = np.ones((128, 1), dtype=ml_dtypes.bfloat16)
    ones1 = np.ones((1, 128), dtype=np.float16)

    in_maps = []
    for c in range(NCORES):
        b, hg = divmod(c, HG)
        qcols = slice(hg * GQ, (hg + 1) * GQ)
        kcols = slice(D + hg * GQ, D + (hg + 1) * GQ)
        vcols = slice(2 * D + hg * GQ, 2 * D + (hg + 1) * GQ)
        wqk_c = np.ascontiguousarray(
            np.concatenate([Wqkv[:, qcols], Wqkv[:, kcols]], axis=1)
        ).astype(ml_dtypes.bfloat16).reshape(16, 128, 2 * GQ)
        wv_c = np.ascontiguousarray(Wqkv[:, vcols]).astype(ml_dtypes.bfloat16).reshape(16, 128, GQ)
        wo_c = np.ascontiguousarray(Wo[hg * GQ:(hg + 1) * GQ, :]).astype(ml_dtypes.bfloat16).reshape(4, 128, D)
        bqk_c = np.concatenate([bqkv[qcols], bqkv[kcols]]).reshape(8, 128).T
        bqk_sw = np.concatenate([bqk_c[64:128], bqk_c[0:64]], axis=0)
        bv_c = bqkv[vcols].reshape(1, GQ)
        cosT, sin_rot = tabs[b]
        in_maps.append({
            "xt": xts[b], "wqk": wqk_c, "wv": wv_c, "wo": wo_c,
            "bqkt": np.ascontiguousarray(bqk_c), "bqkt_sw": np.ascontiguousarray(bqk_sw),
            "bv": np.ascontiguousarray(bv_c),
            "cost": cosT, "sinrt": sin_rot, "maskd": mask,
            "onesb": onesb, "ones1": ones1,
        })
    return in_maps


def kernel(hidden_states, position_ids, Wqkv, bqkv, Wo, bo, _reps=1):
    bo = np.asarray(bo, dtype=np.float32)
    in_maps = _make_in_maps({
        "hidden_states": hidden_states, "position_ids": position_ids,
        "Wqkv": Wqkv, "bqkv": bqkv, "Wo": Wo, "bo": bo,
    })
    nc = _get_nc(_reps)
    res = run_bass_kernel_spmd(nc, in_maps, core_ids=list(range(NCORES)))

    out = np.empty((B, S, D), dtype=np.float32)
    for b in range(B):
        acc = res.results[b * HG]["outt"].reshape(D, S).astype(np.float32).copy()
        for hg in range(1, HG):
            acc += res.results[b * HG + hg]["outt"].reshape(D, S).astype(np.float32)
        out[b] = acc.T + bo[None, :]
    return out
